# revision 101
# baseline (speedup 1.0000x reference)
"""LongFormer sliding-window attention on 8 Trainium2 NeuronCores.

Sharding: batch*heads data-parallel. 24 (batch, head) pairs -> 8 cores,
each core owns one batch (core//4) and 3 consecutive heads (3*(core%4)).
No collectives.

v2 layout strategy (all-fp16 compute, fp32 PSUM accumulation):
  - x arrives pre-transposed AND pre-tiled from host as fp16
    [128, 6*4096] (kt-plane-major), one DMA per 512-token stripe.
  - Q/K projected with the weight matrix stationary into transposed
    layout qkT [128, 3*4096]: group 0 = Q heads01, group 1 = K heads01,
    group 2 = [Q h2 | K h2] packed M=128 (rows 0:64 = qT h2, rows
    64:128 = kT h2).  A small SBUF->SBUF DMA per stripe relocates the
    qT h2 rows to partitions 64:128 of a separate tile so the h2 score
    matmuls see both operands at the same base partition (fp16 keeps
    every matmul at 1 cycle/row regardless of N; fp8 fails the 2e-2
    tolerance, tested).
  - V projected with x stationary into natural [token, dim] layout,
    3 heads x (64 dims + ones/bias col) = 195 cols, two row-tiles per
    PSUM bank; the ones column provides the softmax denominator through
    the PV matmul, and the bias+ones row is added by the PSUM->SBUF
    DVE move (tensor_tensor add against a host-replicated row).
  - scores are computed TRANSPOSED: scoresT[k, qi] with lhsT=kT tile
    [64,128] fp16, rhs=qT chunk [64, N] fp16. Per (chunk, head) the 6
    key-tile score blocks pack into 3 PSUM banks (384|512|384 cols) so
    softmax-exp is 3 Activation instructions into one packed fp16 E
    tile [128, 1280]; the A/C banks carry the two 65-col PV
    accumulators in their tails so each chunk-head uses exactly 3
    banks and the shared 8-buf PSUM pool rotates deeply.
  - band triangle masks applied multiplicatively on E with two strided
    2-block DVE multiplies (4x DVE mode); masks stored [l,u,l,u] so
    each pair is one stride-256 access pattern.
  - PV: out[qi, d] accumulated over 5 key tiles per 128-query half with
    E 128-col slices stationary, vsb [128,65] moving (unmasked tiles
    first); reciprocal of the ones-column then per-partition scale into
    an fp32 staging tile; output DMA'd per chunk (2 row tiles).
  - schedule: greedy gated software pipeline at head granularity - PV
    lags scores by 2 slots and one stripe-projection unit is emitted
    per slot (skipping every 5th, last 4 held for the tail) so the PE
    never head-of-line blocks on softmax-exp; a few junk matmuls gated
    on the first weight DMA warm the PE p-state during the x wait. The
    last stripe's QK units are split into 256-token halves so the tail
    chunks' scores (and their ACT exps) unlock earlier.
"""

import sys

import numpy as np

sys.path.insert(0, "/opt/trn_rl_repo")

import concourse.bass as bass  # noqa: E402
import concourse.tile as tile  # noqa: E402
from concourse import bacc, mybir  # noqa: E402
from concourse import bass_utils  # noqa: E402

B, S, E = 2, 4096, 768
H, D = 12, 64
W2 = 256            # one-sided window w
C = S // W2         # 16 chunks of 256 queries
HPC = 3             # heads per core
N_CORES = 8

f32 = mybir.dt.float32
f16 = mybir.dt.float16

KT = 6              # 768 = 6 k-tiles of 128
NT = 8              # 4096 = 8 n-tiles of 512
RT = 32             # 4096 = 32 row-tiles of 128
VW = 65 * HPC       # packed v width: 3 heads x (64 dims + ones col)

# E-tile packed column offsets for score tiles t=0..5
ECOL = [0, 128, 384, 640, 896, 1152]
EWID = [128, 256, 256, 256, 256, 128]
# bank -> tiles
BANKS = [(0, 1), (2, 3), (4, 5)]


def _build_body(tc, aps):
    nc = tc.nc
    xt_d, wqk_d, bqk_d, wv_d, wvr_d, masks_d, out_d = aps

    from contextlib import ExitStack
    ctx = ExitStack()
    sb = ctx.enter_context(tc.tile_pool(name="sb", bufs=1))
    e_p = ctx.enter_context(tc.tile_pool(name="ep", bufs=10))
    out_p = ctx.enter_context(tc.tile_pool(name="outp", bufs=2))
    rec_p = ctx.enter_context(tc.tile_pool(name="recp", bufs=8))
    ps = ctx.enter_context(tc.tile_pool(name="ps", bufs=8, space="PSUM"))

    # ---- persistent SBUF tensors ----
    # DMA order: wqk group 0 + xt stripe 0 first so compute starts ASAP.
    # wqk is group-major: [128, g*6*128 + kt*128 + c]
    wqk = sb.tile([128, KT * 384], f16, tag="wqk")
    xt = sb.tile([128, KT * S], f16, tag="xt")       # 48 KiB/part

    def xt_stripe_dma(nt, split=False):
        # one DMA per 512-token stripe: [128, 6, 512] from [128, 6*4096]
        for k0, k1 in ([(0, 3), (3, KT)] if split else [(0, KT)]):
            nc.sync.dma_start(
                xt[:, :].rearrange("p (k n) -> p k n", k=KT)[
                    :, k0:k1, nt * 512:(nt + 1) * 512],
                xt_d.rearrange("p (k n) -> p k n", k=KT)[
                    :, k0:k1, nt * 512:(nt + 1) * 512],
            )

    def wqk_dma(g):
        nc.sync.dma_start(wqk[:, g * 768:(g + 1) * 768],
                          wqk_d[:, g * 768:(g + 1) * 768])

    wqk_dma(0)
    xt_stripe_dma(0, split=True)
    wqk_dma(1)
    # p-state warmup: junk matmuls gated only on the wqk DMA keep the PE
    # busy during the xt stripe-0 wait so real work starts at full clock
    for wi in range(8):
        pw = ps.tile([128, 512], f32, tag="ps", name=f"warm{wi}")
        nc.tensor.matmul(pw[:], wqk[:, 0:128], wqk[:, 0:512],
                         start=True, stop=True)
    bqk = sb.tile([128, 3], f32, tag="bqk")
    nc.sync.dma_start(bqk[:], bqk_d[:])
    xt_stripe_dma(1)
    wqk_dma(2)
    wv = sb.tile([128, KT * 195], f16, tag="wv")
    nc.sync.dma_start(wv[:], wv_d[:])
    # bias+ones row replicated across partitions, laid out to match the
    # two-subtile pv bank pattern (cols 0:195 and 256:451)
    wvr = sb.tile([128, 512], f16, tag="wvr")
    nc.sync.dma_start(wvr[:], wvr_d[:])
    xt_stripe_dma(2)
    masks = sb.tile([128, 512], f16, tag="masks")   # [l, u, l, u]
    nc.sync.dma_start(masks[:], masks_d[:])
    xt_stripe_dma(3)

    qkT = sb.tile([128, 3 * S], f16, tag="qkT")      # 24 KiB/part
    qk2m = sb.tile([128, S], f16, tag="qk2m")        # 8 KiB/part (rows 64:128 used)
    vsb = sb.tile([128, RT * VW], f16, tag="vsb")    # 12.2 KiB/part

    def xslice(kt, lo, n):
        return xt[:, kt * S + lo: kt * S + lo + n]

    # ---- projection work units for one 512-token stripe ----
    def stripe_qk(nt, g, half=None):
        # half=0/1 processes 256 tokens (used to split the last stripe so
        # the tail chunks' scores unlock earlier)
        lo = nt * 512 + (0 if not half else 256)
        n = 512 if half is None else 256
        pq = ps.tile([128, 512], f32, tag="ps", name=f"pq{nt}g{g}h{half}")
        for kt in range(KT):
            nc.tensor.matmul(
                pq[:, 0:n],
                wqk[:, g * 768 + kt * 128: g * 768 + (kt + 1) * 128],
                xslice(kt, lo, n),
                start=(kt == 0), stop=(kt == KT - 1),
            )
        nc.vector.tensor_scalar_add(
            qkT[:, g * S + lo: g * S + lo + n],
            pq[:, 0:n], bqk[:, g:g + 1])
        if g == 2:
            # relocate qT h2 (rows 0:64 of group 2) to partitions 64:128
            nc.sync.dma_start(
                qk2m[64:128, lo: lo + n],
                qkT[0:64, 2 * S + lo: 2 * S + lo + n])

    def stripe_v(nt, half):
        # 2 row tiles per PSUM bank (cols 0:195 and 256:451)
        pv = ps.tile([128, 512], f32, tag="ps", name=f"pv{nt}h{half}")
        for sub in range(2):
            rt = nt * 4 + half * 2 + sub
            for kt in range(KT):
                nc.tensor.matmul(
                    pv[:, sub * 256: sub * 256 + VW],
                    xslice(kt, rt * 128, 128),
                    wv[:, kt * 195:(kt + 1) * 195],
                    start=(kt == 0), stop=(kt == KT - 1),
                )
        rt0 = nt * 4 + half * 2
        # bias + ones column added during the PSUM->SBUF move
        nc.vector.tensor_tensor(
            vsb[:, rt0 * VW: rt0 * VW + 2 * VW].rearrange(
                "p (a n) -> p a n", a=2),
            pv[:].rearrange("p (a n) -> p a n", a=2)[:, :, 0:VW],
            wvr[:].rearrange("p (a n) -> p a n", a=2)[:, :, 0:VW],
            mybir.AluOpType.add)

    # ---- banded attention for one chunk ----
    # head h slices: h in {0,1}: qT = group0 rows 64h..64h+64, kT = group1
    # same rows; h=2: qT = qk2m rows 64:128, kT = group2 rows 64:128.
    def q_slice(h, lo, n):
        if h < 2:
            return qkT[64 * h: 64 * h + 64, lo: lo + n]
        return qk2m[64:128, lo: lo + n]

    def k_slice(h, lo, n):
        if h < 2:
            return qkT[64 * h: 64 * h + 64, S + lo: S + lo + n]
        return qkT[64:128, 2 * S + lo: 2 * S + lo + n]

    def do_chunk(c):
        tmin = 2 if c == 0 else 0
        tmax = 3 if c == C - 1 else 5
        return tmin, tmax

    # strided 2-block AP helper: [p, {off:off+128, off+256:off+384}]
    def _pair(t, off):
        return t[:].rearrange("p (a n) -> p a n", n=128)[:, off // 128: off // 128 + 3: 2, :]

    chunk_ctx = {}

    def chunk_scores(c, hi):
        tmin, tmax = do_chunk(c)
        et = e_p.tile([128, 1280], f16, tag="et", name=f"et{c}h{hi}")
        # score banks A/C carry a 65-col po accumulator in their unused
        # tail; at boundaries the skipped bank is a plain po bank so
        # every chunk-head uses exactly 3 PSUM banks.
        banks = {}
        for bi, (ta, tb) in enumerate(BANKS):
            ts_b = [t for t in (ta, tb) if tmin <= t <= tmax]
            pt = ps.tile([128, 512], f32, tag="ps", name=f"sc{c}h{hi}b{bi}")
            banks[bi] = (pt, ts_b)
            if not ts_b:
                continue
            base = ECOL[ts_b[0]]
            for t in ts_b:
                kt_abs = 2 * (c - 1) + t
                qlo, qn = (0, 128) if t == 0 else ((128, 128) if t == 5 else (0, 256))
                nc.tensor.matmul(
                    pt[:, ECOL[t] - base: ECOL[t] - base + qn],
                    k_slice(hi, kt_abs * 128, 128),
                    q_slice(hi, c * 256 + qlo, qn),
                    start=True, stop=True,
                )
            w = ECOL[ts_b[-1]] + EWID[ts_b[-1]] - base
            nc.scalar.activation(
                et[:, base: base + w], pt[:, 0:w],
                mybir.ActivationFunctionType.Exp, scale=0.125)
        # triangle masks (lower on t0 + t1-half, upper on t4-half + t5)
        if tmin == 0:
            nc.vector.tensor_mul(_pair(et, 0), _pair(et, 0), _pair(masks, 0))
        if tmax == 5:
            nc.vector.tensor_mul(_pair(et, 896), _pair(et, 896), _pair(masks, 128))
        chunk_ctx[(c, hi)] = (et, banks)

    def chunk_pv(c, hi):
        tmin, tmax = do_chunk(c)
        et, banks = chunk_ctx.pop((c, hi))
        if c % 4 == 0 and hi == 0:
            chunk_pv.ots = out_p.tile([128, 1536], f32, tag="ot", name=f"ot{c // 4}")
        ots = chunk_pv.ots
        for qh in range(2):
            ts_l = [t for t in range(tmin, tmax + 1)
                    if (t <= 4 if qh == 0 else t >= 1)]
            # unmasked tiles first so PV can start before the
            # triangle-mask multiplies finish (masked: qh0 t0/t4, qh1 t1/t5)
            ts_l.sort(key=lambda t: (t in ((0, 4) if qh == 0 else (1, 5)), t))
            pbank, pts = banks[0 if qh == 0 else 2]
            pcol = 384 if pts else 0
            po = pbank[:, pcol: pcol + 65]
            for i, t in enumerate(ts_l):
                kt_abs = 2 * (c - 1) + t
                ecol = ECOL[t] if (qh == 0 or t == 5) else ECOL[t] + 128
                nc.tensor.matmul(
                    po[:],
                    et[:, ecol: ecol + 128],
                    vsb[:, kt_abs * VW + hi * 65: kt_abs * VW + (hi + 1) * 65],
                    start=(i == 0), stop=(i == len(ts_l) - 1),
                )
            rec = rec_p.tile([128, 1], f32, tag="rec")
            nc.vector.reciprocal(rec[:], po[:, 64:65])
            rt8 = (2 * c + qh) % 8
            nc.vector.tensor_scalar_mul(
                ots[:, rt8 * 192 + hi * 64: rt8 * 192 + (hi + 1) * 64],
                po[:, 0:64], rec[:])
        if hi == HPC - 1:
            q = (2 * c) % 8
            nc.sync.dma_start(
                out_d.rearrange("(r p) e -> p r e", p=128)[:, 2 * c: 2 * c + 2, :],
                ots[:, q * 192:(q + 2) * 192].rearrange("p (r e) -> p r e", r=2))

    # ---- schedule ----
    # Greedy gated software pipeline at head granularity: PV lags scores
    # by 2 slots; one stripe-projection unit is emitted per slot as PE
    # filler (so PE never head-of-line blocks on softmax-exp), with gate
    # requirements draining the stripe queue early when needed.
    squeue = [("qk", 0, 0), ("qk", 0, 1), ("qk", 1, 1), ("qk", 1, 0),
              ("qk", 0, 2), ("qk", 1, 2),
              ("v", 0, 0), ("v", 0, 1), ("v", 1, 0), ("v", 1, 1)]
    for nt in range(2, NT - 1):
        squeue += [("qk", nt, 0), ("qk", nt, 1), ("qk", nt, 2),
                   ("v", nt, 0), ("v", nt, 1)]
    squeue += [("qkh", (7, 0), 0), ("qkh", (7, 0), 1), ("qkh", (7, 0), 2),
               ("v", 7, 0),
               ("qkh", (7, 1), 0), ("qkh", (7, 1), 1), ("qkh", (7, 1), 2),
               ("v", 7, 1)]
    sq_pos = [0]
    xt_next = [4]

    def emit_stripe_unit():
        if sq_pos[0] >= len(squeue):
            return
        kind, nt, x = squeue[sq_pos[0]]
        sq_pos[0] += 1
        if kind == "qk":
            stripe_qk(nt, x)
        elif kind == "qkh":
            stripe_qk(nt[0], x, half=nt[1])
            nt = nt[0]
        else:
            stripe_v(nt, x)
        # keep xt DMAs ~1 stripe ahead of the queue consumption
        need_nt = nt + 2
        while xt_next[0] <= min(need_nt, NT - 1):
            xt_stripe_dma(xt_next[0])
            xt_next[0] += 1

    def ensure(pred_idx):
        while sq_pos[0] <= pred_idx:
            emit_stripe_unit()

    def q_index(kind, nt, x):
        return squeue.index((kind, nt, x))

    def qk_idx(m, g, c):
        # chunk c's k-span ends at token (c+2)*256; for stripe 7 that
        # may need only the first half
        if m < NT - 1:
            return q_index("qk", m, g)
        h = 0 if (c + 2) * 256 <= NT * 512 - 256 else 1
        return q_index("qkh", (NT - 1, h), g)

    def sc_gate(c, hi):
        m = min((c + 1) // 2, NT - 1)
        if hi < 2:
            return max(qk_idx(c // 2, 0, c), qk_idx(m, 1, c))
        return qk_idx(m, 2, c)

    def pv_gate(c):
        mv = min((2 * c + 3) // 4, NT - 1)
        h = 1 if (2 * c + 3) % 4 >= 2 else 0
        return q_index("v", mv, h)

    # chunks 0/1 defer head 2 (its qk-group-2 stripe units gate late)
    sc_stream = [(0, 0), (0, 1), (1, 0), (1, 1), (0, 2), (1, 2)] + \
        [(c, hi) for c in range(2, C) for hi in range(HPC)]
    for i in range(len(sc_stream) + 2):
        if i < len(sc_stream):
            c, hi = sc_stream[i]
            ensure(sc_gate(c, hi))
            chunk_scores(c, hi)
        if i >= 2:
            c2, h2 = sc_stream[i - 2]
            ensure(pv_gate(c2))
            chunk_pv(c2, h2)
        # spread the 40 stripe units over all slots so neither the
        # filler phase (PE-bound) nor the post-queue phase (ACT-bound)
        # dominates: emit only when behind the uniform pace line
        # (gate-forced drains put us ahead; then we skip)
        if i % 5 != 3 and not (len(squeue) - sq_pos[0] <= 4
                               and i < len(sc_stream) - 6):
            emit_stripe_unit()
    ctx.close()


def build_program():
    nc = bacc.Bacc("TRN2", target_bir_lowering=False, debug=False)
    xt_d = nc.dram_tensor("xt", [128, KT * S], f16, kind="ExternalInput").ap()
    wqk_d = nc.dram_tensor("wqk", [128, KT * 384], f16, kind="ExternalInput").ap()
    bqk_d = nc.dram_tensor("bqk", [128, 3], f32, kind="ExternalInput").ap()
    wv_d = nc.dram_tensor("wv", [128, KT * 195], f16, kind="ExternalInput").ap()
    wvr_d = nc.dram_tensor("wvr", [128, 512], f16, kind="ExternalInput").ap()
    masks_d = nc.dram_tensor("masks", [128, 512], f16, kind="ExternalInput").ap()
    out_d = nc.dram_tensor("out", [S, 192], f32, kind="ExternalOutput").ap()
    with tile.TileContext(nc) as tc:
        _build_body(tc, (xt_d, wqk_d, bqk_d, wv_d, wvr_d, masks_d, out_d))
    nc.compile()
    return nc


def make_in_maps(hidden_states, Wq, bq, Wk, bk, Wv, bv):
    hs = np.asarray(hidden_states, np.float32)
    Wq = np.asarray(Wq, np.float32)
    Wk = np.asarray(Wk, np.float32)
    Wv = np.asarray(Wv, np.float32)
    bq = np.asarray(bq, np.float32)
    bk = np.asarray(bk, np.float32)
    bv = np.asarray(bv, np.float32)

    # x^T tiled to SBUF layout [128, kt*4096]
    xts = []
    for b in range(B):
        xT = hs[b].T.astype(np.float16)                   # [768, 4096]
        xts.append(np.ascontiguousarray(
            xT.reshape(KT, 128, S).transpose(1, 0, 2).reshape(128, KT * S)))

    mask_l = np.tril(np.ones((128, 128), np.float16))
    mask_u = np.triu(np.ones((128, 128), np.float16))
    masks = np.ascontiguousarray(
        np.concatenate([mask_l, mask_u, mask_l, mask_u], axis=1))  # [128, 512]

    in_maps = []
    for core in range(N_CORES):
        b = core // 4
        h0 = HPC * (core % 4)
        # group cols: g0 = Wq h01 (128), g1 = Wk h01 (128), g2 = [Wq h2 | Wk h2]
        wqk_full = np.concatenate(
            [Wq[:, h0 * 64:(h0 + 2) * 64], Wk[:, h0 * 64:(h0 + 2) * 64],
             Wq[:, (h0 + 2) * 64:(h0 + 3) * 64], Wk[:, (h0 + 2) * 64:(h0 + 3) * 64]],
            axis=1).astype(np.float16)                    # [768, 384]
        # group-major SBUF layout: [128, g, kt, 128]
        wqk = np.ascontiguousarray(
            wqk_full.reshape(KT, 128, 3, 128).transpose(1, 2, 0, 3).reshape(128, KT * 384))
        bqk = np.zeros((128, 3), np.float32)
        bqk[:, 0] = bq[h0 * 64:(h0 + 2) * 64]
        bqk[:, 1] = bk[h0 * 64:(h0 + 2) * 64]
        bqk[0:64, 2] = bq[(h0 + 2) * 64:(h0 + 3) * 64]
        bqk[64:128, 2] = bk[(h0 + 2) * 64:(h0 + 3) * 64]
        wv_full = np.zeros((E, 195), np.float16)
        wvr_row = np.zeros(512, np.float16)
        for i in range(HPC):
            wv_full[:, 65 * i: 65 * i + 64] = Wv[:, (h0 + i) * 64:(h0 + i + 1) * 64]
            for off in (0, 256):
                wvr_row[off + 65 * i: off + 65 * i + 64] = \
                    bv[(h0 + i) * 64:(h0 + i + 1) * 64]
                wvr_row[off + 65 * i + 64] = 1.0
        wv = np.ascontiguousarray(
            wv_full.reshape(KT, 128, 195).transpose(1, 0, 2).reshape(128, KT * 195))
        in_maps.append({
            "xt": xts[b],
            "wqk": wqk,
            "bqk": bqk,
            "wv": wv,
            "wvr": np.ascontiguousarray(np.tile(wvr_row, (128, 1))),
            "masks": masks,
        })
    return in_maps


_NC_CACHE = None


def kernel(hidden_states, Wq, bq, Wk, bk, Wv, bv):
    global _NC_CACHE
    if _NC_CACHE is None:
        _NC_CACHE = build_program()
    nc = _NC_CACHE
    in_maps = make_in_maps(hidden_states, Wq, bq, Wk, bk, Wv, bv)
    res = bass_utils.run_bass_kernel_spmd(nc, in_maps, core_ids=list(range(N_CORES)))
    out = np.zeros((B, S, H * D), np.float32)
    for core in range(N_CORES):
        b = core // 4
        h0 = HPC * (core % 4)
        out[b, :, h0 * 64:(h0 + HPC) * 64] = res.results[core]["out"]
    return out
